# revision 8
# baseline (speedup 1.0000x reference)
"""Trainium2 Bass kernel for AcceleratedAttentionPool1d (v3).

Algebra: only the CENTER row of each window's attention survives, so per
output position s:
  qtok = (Wq @ xp + bq)/sqrt(24)            (scale folded into weights)
  energy[s, j] = <qtok[:, s+4], qtok[:, s0+j]>  over a 9-wide band
  attn = softmax(energy) over the band
  out[:, s] = (Wo/9) @ (sum_j attn[s,j] xp[:, s0+j]) + bo/9
The output projection folds into the V side: wxh[c][h, f] = sum_e
xp[e, s0+h]*(Wo/9)[f, e]; fin[f, c] = sum_h wxh[h, f] * at[h, c].

Sharding: data-parallel over batch; B=8 batches on 8 cores.

v3 structure (vs v2):
 - All intermediate PSUM in bf16: matmul e_i halves land in separate
   bf16 psum slots; evictions read bf16 psum (DVE 2x mode) and fuse the
   halves' add + bias via scalar_tensor_tensor.
 - The -1e30 band mask is ADDED IN PSUM by an extra matmul
   (lhsT=identity, rhs=mask const) before the energy matmuls -- no
   vector mask op.
 - Softmax normalization folded into the attention transpose: r=1/sums
   is written onto a diagonal D (tensor_scalar_mul of identity), and
   the transpose is matmul(lhsT=A, rhs=D) => at[h,c]=A[c,h]*r[c].
 - sums via bf16 tensor_reduce (2x/4x DVE), recip in bf16.
 - Inputs sliced fine (4 pieces per E-half) across sync+gpsimd rings so
   the first qtok matmul starts as early as possible; consts packed
   into 2 blobs on the scalar ring.
 - Output stored per chunk-pair ([128, 240, 2] bf16, host unpacks
   [128, S, 2]) on the sync ring as soon as each pair's fin evicts.
"""

import numpy as np
import ml_dtypes

import concourse.bass as bass
import concourse.mybir as mybir
import concourse.tile as tile
from concourse import bacc
from concourse.bass import ts
from concourse.bass_utils import run_bass_kernel_spmd

F32 = mybir.dt.float32
BF16 = mybir.dt.bfloat16

B, E, S = 8, 256, 2048
KERNEL = 9
PAD = KERNEL // 2
SP = S + 2 * PAD  # 2056
C = 120  # output positions per chunk
H = 128  # halo width
NCHUNK = 18  # 17 full strides + 1 overlapping tail chunk
SOFT_G = 3  # chunks per fused softmax subgroup
NSG = NCHUNK // SOFT_G
NEG = -1.0e30

T_CH = [(0, 512), (512, 512), (1024, 512), (1536, 512), (2048, 8)]
X_PC = [(0, 512), (512, 512), (1024, 512), (1536, 520)]  # dma pieces


def _cs(c: int) -> int:
    return 120 * c if c < NCHUNK - 1 else S - C  # last chunk overlaps


def build_nc() -> bass.Bass:
    nc = bacc.Bacc("TRN2", target_bir_lowering=False)

    x0_d = nc.dram_tensor("x0", [128, SP], BF16, kind="ExternalInput")
    x1_d = nc.dram_tensor("x1", [128, SP], BF16, kind="ExternalInput")
    wqt_d = nc.dram_tensor("wqt", [128, 2, E], BF16, kind="ExternalInput")
    # cblob: [:, 0:2, :]=wot, [:, 2, 0:128]=mask, [:, 2, 128:256]=ident
    cblob_d = nc.dram_tensor("cblob", [128, 3, E], BF16, kind="ExternalInput")
    bvec_d = nc.dram_tensor("bvec", [128, 4], F32, kind="ExternalInput")
    out_d = nc.dram_tensor("out", [128, S, 2], BF16, kind="ExternalOutput")

    with tile.TileContext(nc) as tc:
        with (
            tc.tile_pool(name="const", bufs=1) as const,
            tc.tile_pool(name="work", bufs=4) as work,
            tc.tile_pool(name="grp", bufs=3) as grp,
            tc.tile_pool(name="ps", bufs=1, space="PSUM") as ps,
        ):
            # ---- DMA intake.
            # sync ring: wqt then x0 pieces (and later all output stores)
            # gpsimd ring: x1 pieces; scalar ring: const blobs.
            wqt_t = const.tile([128, 2, E], BF16)
            nc.sync.dma_start(wqt_t, wqt_d[:, :, :])
            x0_t = const.tile([128, SP], BF16)
            x1_t = const.tile([128, SP], BF16)
            for p0, w in X_PC:
                nc.sync.dma_start(x0_t[:, p0 : p0 + w], x0_d[:, p0 : p0 + w])
            for p0, w in X_PC:
                nc.gpsimd.dma_start(x1_t[:, p0 : p0 + w], x1_d[:, p0 : p0 + w])
            cblob_t = const.tile([128, 3, E], BF16)
            nc.scalar.dma_start(cblob_t, cblob_d[:, :, :])
            bvec_t = const.tile([128, 4], F32)
            nc.scalar.dma_start(bvec_t, bvec_d[:, :])
            wot_t = cblob_t[:, 0:2, :]
            mask_t = cblob_t[:, 2, 0:128]
            id_t = cblob_t[:, 2, 128:256]

            xs = [x0_t, x1_t]
            qtok0 = const.tile([128, SP], BF16)
            qtok1 = const.tile([128, SP], BF16)
            qtoks = [qtok0, qtok1]
            wxh_t = const.tile([128, NCHUNK, E], BF16)

            # persistent per-parity tiles; pad rows zeroed once
            A_slots = [
                const.tile([128, SOFT_G, H], BF16, name=f"A{i}") for i in range(2)
            ]
            for t in A_slots:
                nc.gpsimd.memset(t[96:128, :, :].bitcast(mybir.dt.uint32), 0)

            fo_tiles = {}
            pf_state = {}

            def emit_wxh(c):
                ci = c % 2
                if ci == 0:
                    emit_wxh.pwx = ps.tile(
                        [128, 2, E], F32, tag="pswx", bufs=1, name="pwx"
                    )
                pwx = emit_wxh.pwx
                s0 = _cs(c)
                for e_i in range(2):
                    nc.tensor.matmul(
                        pwx[:, ci, :],
                        lhsT=xs[e_i][:, s0 : s0 + H],
                        rhs=wot_t[:, e_i, :],
                        start=(e_i == 0),
                        stop=(e_i == 1),
                    )
                if ci == 1:
                    nc.scalar.copy(wxh_t[:, c - 1 : c + 1, :], pwx)

            def emit_q(t_i):
                t0, w = T_CH[t_i]
                for f_i in range(2):
                    pq = ps.tile([128, 512], F32, tag="psq", bufs=2, name="pq")
                    for e_i in range(2):
                        nc.tensor.matmul(
                            pq[:, :w],
                            lhsT=wqt_t[:, e_i, ts(f_i, 128)],
                            rhs=xs[e_i][:, t0 : t0 + w],
                            start=(e_i == 0),
                            stop=(e_i == 1),
                        )
                    nc.vector.tensor_scalar_add(
                        qtoks[f_i][:, t0 : t0 + w], pq[:, :w], bvec_t[:, f_i : f_i + 1]
                    )

            def emit_e(k):
                pe_ = ps.tile([C, SOFT_G, H], F32, tag="pse", bufs=2, name="pe_")
                emit_e.pe[k] = pe_
                for gi in range(SOFT_G):
                    c = SOFT_G * k + gi
                    s0 = _cs(c)
                    nc.tensor.matmul(
                        pe_[:, gi, :],
                        lhsT=id_t[:, 0:C],
                        rhs=mask_t[:, :],
                        start=True,
                        stop=False,
                    )
                    for f_i in range(2):
                        nc.tensor.matmul(
                            pe_[:, gi, :],
                            lhsT=qtoks[f_i][:, s0 + PAD : s0 + PAD + C],
                            rhs=qtoks[f_i][:, s0 : s0 + H],
                            start=False,
                            stop=(f_i == 1),
                        )

            emit_e.pe = {}

            def emit_fin(c, at_ap):
                g = c // 2
                ci = c % 2
                if ci == 0:
                    pf_state[g] = ps.tile(
                        [128, 2, 2 * C], F32, tag="psf", bufs=2, name="pf"
                    )
                pf = pf_state[g]
                for f_i in range(2):
                    nc.tensor.matmul(
                        pf[:, f_i, ci * C : ci * C + C],
                        lhsT=wxh_t[:, c, ts(f_i, 128)],
                        rhs=at_ap,
                        start=True,
                        stop=True,
                    )
                if ci == 1:
                    fo = grp.tile([128, 2 * C, 2], BF16, tag="fo", name="fo")
                    fo_tiles[g] = fo
                    for f_i in range(2):
                        nc.scalar.activation(
                            fo[:, :, f_i],
                            pf[:, f_i, :],
                            mybir.ActivationFunctionType.Identity,
                            bias=bvec_t[:, 2 + f_i : 3 + f_i],
                            scale=1.0,
                        )
                    if g < 8:
                        nc.sync.dma_start(
                            out_d[:, 240 * g : 240 * g + 240, :], fo
                        )
                    else:
                        nc.sync.dma_start(out_d[:, 1920:2040, :], fo[:, 0:C, :])
                        nc.sync.dma_start(
                            out_d[:, 2040:2048, :], fo[:, 232:240, :]
                        )

            def emit_tf(k):
                pe_ = emit_e.pe.pop(k)
                A = A_slots[k % 2]
                sums = work.tile([128, SOFT_G], BF16, tag="sums", name="sums")
                nc.scalar.activation(
                    A[:C, :, :], pe_, mybir.ActivationFunctionType.Exp
                )
                r = work.tile([128, SOFT_G], F32, tag="r", name="r")
                with nc.allow_low_precision(reason="9-term softmax sums in bf16"):
                    nc.vector.tensor_reduce(
                        sums[:C, :],
                        A[:C, :, :],
                        axis=mybir.AxisListType.X,
                        op=mybir.AluOpType.add,
                    )
                nc.vector.reciprocal(r[:C, :], sums[:C, :])
                for gi in range(SOFT_G):
                    nc.vector.tensor_scalar_mul(
                        A[0:C, gi, :], A[0:C, gi, :], r[0:C, gi : gi + 1]
                    )
                pat = ps.tile([128, SOFT_G, 128], BF16, tag="psat", bufs=1, name="pat")
                for gi in range(SOFT_G):
                    nc.tensor.transpose(pat[:, gi, :], A[:, gi, :], id_t)
                at = work.tile([128, SOFT_G, 128], BF16, tag="at", name="at")
                nc.vector.tensor_copy(at, pat)
                for gi in range(SOFT_G):
                    emit_fin(SOFT_G * k + gi, at[:, gi, 0:C])

            # ---- interleaved schedule (see docstring)
            emit_q(0)
            emit_e(0)
            for c in range(0, 4):
                emit_wxh(c)
            emit_q(1)
            emit_tf(0)
            emit_e(1)
            for c in range(4, 8):
                emit_wxh(c)
            emit_q(2)
            emit_tf(1)
            emit_e(2)
            emit_e(3)
            for c in range(8, 12):
                emit_wxh(c)
            emit_q(3)
            emit_tf(2)
            for c in range(12, 16):
                emit_wxh(c)
            emit_q(4)
            emit_tf(3)
            emit_e(4)
            emit_wxh(16)
            emit_wxh(17)
            emit_e(5)
            emit_tf(4)
            emit_tf(5)

    nc.compile()
    return nc


def make_in_maps(x, Wq, bq, Wo, bo):
    x = np.asarray(x, dtype=np.float32)
    Wq = np.asarray(Wq, dtype=np.float32)
    bq = np.asarray(bq, dtype=np.float32)
    Wo = np.asarray(Wo, dtype=np.float32)
    bo = np.asarray(bo, dtype=np.float32)

    bf = ml_dtypes.bfloat16
    f = 1.0 / np.sqrt(np.sqrt(E) * 1.5)  # 1/sqrt(24) folded into Wq, bq
    wqt = np.ascontiguousarray(
        (Wq * f).T.reshape(2, 128, E).transpose(1, 0, 2)
    ).astype(bf)
    wot = np.ascontiguousarray(
        (Wo / KERNEL).T.reshape(2, 128, E).transpose(1, 0, 2)
    ).astype(bf)

    # cblob: [:, 0:2, :]=wot, [:, 2, 0:128]=mask, [:, 2, 128:256]=ident
    cblob = np.zeros((128, 3, E), dtype=bf)
    cblob[:, 0:2, :] = wot
    mask = np.full((128, 128), NEG, dtype=np.float32)
    for c in range(128):
        mask[c, c : min(c + KERNEL, 128)] = 0.0
    cblob[:, 2, 0:128] = mask.astype(bf)
    cblob[:, 2, 128:256] = np.eye(128, dtype=bf)

    bvec = np.zeros((128, 4), dtype=np.float32)
    bvec[:, 0:2] = (bq * f).reshape(2, 128).T
    bvec[:, 2:4] = (bo / KERNEL).reshape(2, 128).T

    in_maps = []
    for b in range(B):
        xp = np.zeros((E, SP), dtype=np.float32)
        xp[:, PAD : PAD + S] = x[b]
        xpb = xp.astype(bf)
        in_maps.append(
            dict(
                x0=np.ascontiguousarray(xpb[0:128]),
                x1=np.ascontiguousarray(xpb[128:256]),
                wqt=wqt,
                cblob=cblob,
                bvec=bvec,
            )
        )
    return in_maps


_NC_CACHE = {}


def kernel(x, Wq, bq, Wo, bo):
    res = kernel_with_results(x, Wq, bq, Wo, bo)
    outs = []
    for r in res.results:
        o = np.asarray(r["out"])  # [128, S, 2] bf16
        outs.append(
            o.transpose(2, 0, 1).reshape(E, S).astype(np.float32)
        )
    return np.stack(outs)


def kernel_with_results(x, Wq, bq, Wo, bo, trace=False, **kwargs):
    in_maps = make_in_maps(x, Wq, bq, Wo, bo)
    if "nc" not in _NC_CACHE:
        _NC_CACHE["nc"] = build_nc()
    return run_bass_kernel_spmd(
        _NC_CACHE["nc"], in_maps, core_ids=list(range(B)), trace=trace, **kwargs
    )


# revision 16
# speedup vs baseline: 1.2584x; 1.2584x over previous
"""Trainium2 Bass kernel for AcceleratedAttentionPool1d (v4).

Algebra: only the CENTER row of each window's attention survives, so per
output position s:
  qtok = (Wq @ xp + bq)/sqrt(24)            (scale folded into weights)
  energy[s, j] = <qtok[:, s+4], qtok[:, s0+j]>  over a 9-wide band
  attn = softmax(energy) over the band
  out[:, s] = (Wo/9) @ (sum_j attn[s,j] xp[:, s0+j]) + bo/9
The output projection folds into the V side: wxh[c][h, f] = sum_e
xp[e, s0+h]*(Wo/9)[f, e]; fin[f, c] = sum_h wxh[h, f] * at[h, c].

Sharding: data-parallel over batch; B=8 batches on 8 cores.

v4 structure (learned from v2/v3 traces):
 - PE is LDWEIGHTS-bound (~110ns floor per matmul): band mask is added
   in PSUM by ONE merged matmul per softmax subgroup (lhsT=id,
   rhs=mask3 [128,384]) instead of a vector op or 3 matmuls.
 - Emit order puts qtok chunks + energy matmuls as early as their data
   allows, defers transpose+fin PE work, and uses wxh as PE filler, so
   the tail softmax chains overlap compute instead of draining serially.
 - Softmax engine chain (exp scalar -> reduce/recip vector -> norm
   GPSIMD tensor_tensor, which IS legal on TRN2 for SBUF-only ops) is
   emitted right after its energy matmuls; 3 A slots decouple parity.
 - Evictions split: qtok+at+reduce/recip on vector, exp+fin+5 wxh on
   scalar, 4 wxh on vector, norm on gpsimd.
 - Input: x0 pieces on sync ring, x1 pieces on scalar ring, consts on
   gpsimd ring; output stored per chunk-pair on sync as soon as ready.

Dead ends (do not retry): gpsimd InstPool (Pool engine illegal for
InstPool on TRN2 -- DVE only); gpsimd reading PSUM (BIR verifier);
DMA from PSUM (bass assert); matmul bf16 PSUM out (TRN3 only);
tensor_scalar bf16 norm (no DVE fast mode; slower than broadcast TT);
bf16 tensor_reduce (no speedup, worse precision); strided fo[:, :, f]
eviction (slower scalar ACT); x pieces on gpsimd SWDGE ring (late +
5us drain).
"""

import numpy as np
import ml_dtypes

import concourse.bass as bass
import concourse.mybir as mybir
import concourse.tile as tile
from concourse import bacc
from concourse.bass import ts
from concourse.bass_utils import run_bass_kernel_spmd

F32 = mybir.dt.float32
BF16 = mybir.dt.bfloat16

B, E, S = 8, 256, 2048
KERNEL = 9
PAD = KERNEL // 2
SP = S + 2 * PAD  # 2056
C = 120  # output positions per chunk
H = 128  # halo width
NCHUNK = 18  # 17 full strides + 1 overlapping tail chunk
SOFT_G = 3  # chunks per fused softmax subgroup
NSG = NCHUNK // SOFT_G
NEG = -1.0e30

T_CH = [(0, 512), (512, 512), (1024, 512), (1536, 512), (2048, 8)]
X_PC = [(0, 512), (512, 512), (1024, 512), (1536, 520)]  # dma pieces
WXH_VEC = {1, 3, 5, 7}  # wxh pairs evicted on vector (rest scalar)


def _cs(c: int) -> int:
    return 120 * c if c < NCHUNK - 1 else S - C  # last chunk overlaps


def build_nc() -> bass.Bass:
    nc = bacc.Bacc("TRN2", target_bir_lowering=False)

    x0_d = nc.dram_tensor("x0", [128, SP], BF16, kind="ExternalInput")
    x1_d = nc.dram_tensor("x1", [128, SP], BF16, kind="ExternalInput")
    wqt_d = nc.dram_tensor("wqt", [128, 2, E], BF16, kind="ExternalInput")
    # cblob flat: [0:512]=wot, [512:896]=mask3, [896:1024]=ident
    cblob_d = nc.dram_tensor("cblob", [128, 1024], BF16, kind="ExternalInput")
    bvec_d = nc.dram_tensor("bvec", [128, 4], F32, kind="ExternalInput")
    out_d = nc.dram_tensor("out", [128, 2, S], BF16, kind="ExternalOutput")

    with tile.TileContext(nc) as tc:
        with (
            tc.tile_pool(name="const", bufs=1) as const,
            tc.tile_pool(name="work", bufs=4) as work,
            tc.tile_pool(name="grp", bufs=3) as grp,
            tc.tile_pool(name="ps", bufs=1, space="PSUM") as ps,
        ):
            # ---- DMA intake.
            wqt_t = const.tile([128, 2, E], BF16)
            nc.sync.dma_start(wqt_t, wqt_d[:, :, :])
            x0_t = const.tile([128, SP], BF16)
            x1_t = const.tile([128, SP], BF16)
            for p0, w in X_PC:
                nc.sync.dma_start(x0_t[:, p0 : p0 + w], x0_d[:, p0 : p0 + w])
            for p0, w in X_PC:
                nc.scalar.dma_start(x1_t[:, p0 : p0 + w], x1_d[:, p0 : p0 + w])
            cblob_t = const.tile([128, 1024], BF16)
            nc.gpsimd.dma_start(cblob_t, cblob_d[:, :])
            bvec_t = const.tile([128, 4], F32)
            nc.gpsimd.dma_start(bvec_t, bvec_d[:, :])
            wot_t = cblob_t[:, 0:512]  # [:, e_i*256:(e_i+1)*256]
            mask3_t = cblob_t[:, 512:896]
            id_t = cblob_t[:, 896:1024]

            xs = [x0_t, x1_t]
            qtok0 = const.tile([128, SP], BF16)
            qtok1 = const.tile([128, SP], BF16)
            qtoks = [qtok0, qtok1]
            wxh_t = const.tile([128, NCHUNK, E], BF16)

            # persistent A slots; pad rows zeroed once
            A_slots = [
                const.tile([128, SOFT_G, H], BF16, name=f"A{i}") for i in range(3)
            ]
            for t in A_slots:
                nc.gpsimd.memset(t[96:128, :, :].bitcast(mybir.dt.uint32), 0)

            fo_tiles = {}
            pf_state = {}
            soft_state = {}

            def emit_wxh(c):
                ci = c % 2
                if ci == 0:
                    emit_wxh.pwx = ps.tile(
                        [128, 2, E], F32, tag="pswx", bufs=1, name="pwx"
                    )
                pwx = emit_wxh.pwx
                s0 = _cs(c)
                for e_i in range(2):
                    nc.tensor.matmul(
                        pwx[:, ci, :],
                        lhsT=xs[e_i][:, s0 : s0 + H],
                        rhs=wot_t[:, ts(e_i, 256)],
                        start=(e_i == 0),
                        stop=(e_i == 1),
                    )
                if ci == 1:
                    if (c // 2) in WXH_VEC:
                        nc.vector.tensor_copy(wxh_t[:, c - 1 : c + 1, :], pwx)
                    else:
                        nc.scalar.copy(wxh_t[:, c - 1 : c + 1, :], pwx)

            def emit_q(t_i):
                t0, w = T_CH[t_i]
                for f_i in range(2):
                    pq = ps.tile([128, 512], F32, tag="psq", bufs=2, name="pq")
                    for e_i in range(2):
                        nc.tensor.matmul(
                            pq[:, :w],
                            lhsT=wqt_t[:, e_i, ts(f_i, 128)],
                            rhs=xs[e_i][:, t0 : t0 + w],
                            start=(e_i == 0),
                            stop=(e_i == 1),
                        )
                    nc.vector.tensor_scalar_add(
                        qtoks[f_i][:, t0 : t0 + w],
                        pq[:, :w],
                        bvec_t[:, f_i : f_i + 1],
                    )

            def emit_e(k):
                pe_ = ps.tile([C, SOFT_G, H], F32, tag="pse", bufs=2, name="pe_")
                emit_e.pe[k] = pe_
                nc.tensor.matmul(
                    pe_[:, :, :],
                    lhsT=id_t[:, 0:C],
                    rhs=mask3_t[:, :],
                    start=True,
                    stop=False,
                )
                for gi in range(SOFT_G):
                    c = SOFT_G * k + gi
                    s0 = _cs(c)
                    for f_i in range(2):
                        nc.tensor.matmul(
                            pe_[:, gi, :],
                            lhsT=qtoks[f_i][:, s0 + PAD : s0 + PAD + C],
                            rhs=qtoks[f_i][:, s0 : s0 + H],
                            start=False,
                            stop=(f_i == 1 and gi == SOFT_G - 1),
                        )

            emit_e.pe = {}

            def emit_soft(k):
                pe_ = emit_e.pe.pop(k)
                A = A_slots[k % 3]
                soft_state[k] = A
                sums = work.tile([128, SOFT_G], F32, tag="sums", name="sums")
                nc.scalar.activation(
                    A[:C, :, :], pe_, mybir.ActivationFunctionType.Exp
                )
                nc.vector.tensor_reduce(
                    sums[:C, :],
                    A[:C, :, :],
                    axis=mybir.AxisListType.X,
                    op=mybir.AluOpType.add,
                )
                r = work.tile([128, SOFT_G], F32, tag="r", name="r")
                nc.vector.reciprocal(r[:C, :], sums[:C, :])
                nc.gpsimd.tensor_tensor(
                    out=A[:C, :, :],
                    in0=A[:C, :, :],
                    in1=r[:C, :, None].to_broadcast((C, SOFT_G, H)),
                    op=mybir.AluOpType.mult,
                )

            def emit_fin(c, at_ap):
                g = c // 2
                ci = c % 2
                if ci == 0:
                    pf_state[g] = ps.tile(
                        [128, 2, 2 * C], F32, tag="psf", bufs=2, name="pf"
                    )
                pf = pf_state[g]
                for f_i in range(2):
                    nc.tensor.matmul(
                        pf[:, f_i, ci * C : ci * C + C],
                        lhsT=wxh_t[:, c, ts(f_i, 128)],
                        rhs=at_ap,
                        start=True,
                        stop=True,
                    )
                if ci == 1:
                    fo = grp.tile([128, 2, 2 * C], BF16, tag="fo", name="fo")
                    fo_tiles[g] = fo
                    for f_i in range(2):
                        nc.scalar.activation(
                            fo[:, f_i, :],
                            pf[:, f_i, :],
                            mybir.ActivationFunctionType.Identity,
                            bias=bvec_t[:, 2 + f_i : 3 + f_i],
                            scale=1.0,
                        )
                    if g < 8:
                        nc.sync.dma_start(
                            out_d[:, :, 240 * g : 240 * g + 240], fo
                        )
                    else:
                        nc.sync.dma_start(out_d[:, :, 1920:2040], fo[:, :, 0:C])
                        nc.sync.dma_start(
                            out_d[:, :, 2040:2048], fo[:, :, 232:240]
                        )

            def emit_tfin(k):
                A = soft_state.pop(k)
                pat = ps.tile(
                    [128, SOFT_G, 128], BF16, tag="psat", bufs=1, name="pat"
                )
                for gi in range(SOFT_G):
                    nc.tensor.transpose(pat[:, gi, :], A[:, gi, :], id_t)
                at = work.tile([128, SOFT_G, 128], BF16, tag="at", name="at")
                nc.vector.tensor_copy(at, pat)
                for gi in range(SOFT_G):
                    emit_fin(SOFT_G * k + gi, at[:, gi, 0:C])

            # ---- schedule: qtok/energy ASAP, softmax chains right after,
            # transpose+fin deferred, wxh as PE filler.
            emit_q(0)
            emit_e(0)
            emit_soft(0)
            emit_wxh(0)
            emit_wxh(1)
            emit_q(1)
            emit_e(1)
            emit_soft(1)
            emit_wxh(2)
            emit_wxh(3)
            emit_q(2)
            emit_e(2)
            emit_soft(2)
            emit_tfin(0)
            emit_e(3)
            emit_soft(3)
            emit_wxh(4)
            emit_wxh(5)
            emit_q(3)
            emit_e(4)
            emit_tfin(1)
            emit_soft(4)
            emit_wxh(6)
            emit_wxh(7)
            emit_q(4)
            emit_e(5)
            emit_wxh(8)
            emit_wxh(9)
            emit_tfin(2)
            emit_soft(5)
            emit_wxh(10)
            emit_wxh(11)
            emit_wxh(12)
            emit_wxh(13)
            emit_tfin(3)
            emit_wxh(14)
            emit_wxh(15)
            emit_wxh(16)
            emit_wxh(17)
            emit_tfin(4)
            emit_tfin(5)

    nc.compile()
    return nc


def make_in_maps(x, Wq, bq, Wo, bo):
    x = np.asarray(x, dtype=np.float32)
    Wq = np.asarray(Wq, dtype=np.float32)
    bq = np.asarray(bq, dtype=np.float32)
    Wo = np.asarray(Wo, dtype=np.float32)
    bo = np.asarray(bo, dtype=np.float32)

    bf = ml_dtypes.bfloat16
    f = 1.0 / np.sqrt(np.sqrt(E) * 1.5)  # 1/sqrt(24) folded into Wq, bq
    wqt = np.ascontiguousarray(
        (Wq * f).T.reshape(2, 128, E).transpose(1, 0, 2)
    ).astype(bf)
    wot = np.ascontiguousarray(
        (Wo / KERNEL).T.reshape(2, 128, E).transpose(1, 0, 2)
    ).astype(bf)

    # cblob flat: [0:512]=wot, [512:896]=mask3 (3x), [896:1024]=ident
    cblob = np.zeros((128, 1024), dtype=bf)
    cblob[:, 0:256] = wot[:, 0, :]
    cblob[:, 256:512] = wot[:, 1, :]
    mask = np.full((128, 128), NEG, dtype=np.float32)
    for c in range(128):
        mask[c, c : min(c + KERNEL, 128)] = 0.0
    for gi in range(3):
        cblob[:, 512 + 128 * gi : 640 + 128 * gi] = mask.astype(bf)
    cblob[:, 896:1024] = np.eye(128, dtype=bf)

    bvec = np.zeros((128, 4), dtype=np.float32)
    bvec[:, 0:2] = (bq * f).reshape(2, 128).T
    bvec[:, 2:4] = (bo / KERNEL).reshape(2, 128).T

    in_maps = []
    for b in range(B):
        xp = np.zeros((E, SP), dtype=np.float32)
        xp[:, PAD : PAD + S] = x[b]
        xpb = xp.astype(bf)
        in_maps.append(
            dict(
                x0=np.ascontiguousarray(xpb[0:128]),
                x1=np.ascontiguousarray(xpb[128:256]),
                wqt=wqt,
                cblob=cblob,
                bvec=bvec,
            )
        )
    return in_maps


_NC_CACHE = {}


def kernel(x, Wq, bq, Wo, bo):
    res = kernel_with_results(x, Wq, bq, Wo, bo)
    outs = []
    for r in res.results:
        o = np.asarray(r["out"])  # [128, 2, S] bf16
        outs.append(o.transpose(1, 0, 2).reshape(E, S).astype(np.float32))
    return np.stack(outs)


def kernel_with_results(x, Wq, bq, Wo, bo, trace=False, **kwargs):
    in_maps = make_in_maps(x, Wq, bq, Wo, bo)
    if "nc" not in _NC_CACHE:
        _NC_CACHE["nc"] = build_nc()
    return run_bass_kernel_spmd(
        _NC_CACHE["nc"], in_maps, core_ids=list(range(B)), trace=trace, **kwargs
    )


# revision 18
# speedup vs baseline: 1.3342x; 1.0603x over previous
"""Trainium2 Bass kernel for AcceleratedAttentionPool1d (v4).

Algebra: only the CENTER row of each window's attention survives, so per
output position s:
  qtok = (Wq @ xp + bq)/sqrt(24)            (scale folded into weights)
  energy[s, j] = <qtok[:, s+4], qtok[:, s0+j]>  over a 9-wide band
  attn = softmax(energy) over the band
  out[:, s] = (Wo/9) @ (sum_j attn[s,j] xp[:, s0+j]) + bo/9
The output projection folds into the V side: wxh[c][h, f] = sum_e
xp[e, s0+h]*(Wo/9)[f, e]; fin[f, c] = sum_h wxh[h, f] * at[h, c].

Sharding: data-parallel over batch; B=8 batches on 8 cores.

v4 structure (learned from v2/v3 traces):
 - PE is LDWEIGHTS-bound (~110ns floor per matmul): band mask is added
   in PSUM by ONE merged matmul per softmax subgroup (lhsT=id,
   rhs=mask3 [128,384]) instead of a vector op or 3 matmuls.
 - Emit order puts qtok chunks + energy matmuls as early as their data
   allows, defers transpose+fin PE work, and uses wxh as PE filler, so
   the tail softmax chains overlap compute instead of draining serially.
 - Softmax engine chain (exp scalar -> reduce/recip vector -> norm
   GPSIMD tensor_tensor, which IS legal on TRN2 for SBUF-only ops) is
   emitted right after its energy matmuls; 3 A slots decouple parity.
 - Evictions split: qtok+at+reduce/recip on vector, exp+fin+5 wxh on
   scalar, 4 wxh on vector, norm on gpsimd.
 - Input: x0 pieces on sync ring, x1 pieces on scalar ring, consts on
   gpsimd ring; output stored per chunk-pair on sync as soon as ready.

Dead ends (do not retry): gpsimd InstPool (Pool engine illegal for
InstPool on TRN2 -- DVE only); gpsimd reading PSUM (BIR verifier);
DMA from PSUM (bass assert); matmul bf16 PSUM out (TRN3 only);
tensor_scalar bf16 norm (no DVE fast mode; slower than broadcast TT);
bf16 tensor_reduce (no speedup, worse precision); strided fo[:, :, f]
eviction (slower scalar ACT); x pieces on gpsimd SWDGE ring (late +
5us drain).
"""

import numpy as np
import ml_dtypes

import concourse.bass as bass
import concourse.mybir as mybir
import concourse.tile as tile
from concourse import bacc
from concourse.bass import ts
from concourse.bass_utils import run_bass_kernel_spmd

F32 = mybir.dt.float32
BF16 = mybir.dt.bfloat16

B, E, S = 8, 256, 2048
KERNEL = 9
PAD = KERNEL // 2
SP = S + 2 * PAD  # 2056
C = 120  # output positions per chunk
H = 128  # halo width
NCHUNK = 18  # 17 full strides + 1 overlapping tail chunk
SOFT_G = 3  # chunks per fused softmax subgroup
NSG = NCHUNK // SOFT_G
NEG = -1.0e30

T_CH = [(0, 256), (256, 256), (512, 512), (1024, 512), (1536, 512), (2048, 8)]
X_PC = [(0, 256), (256, 256), (512, 512), (1024, 512), (1536, 520)]  # dma pieces
WXH_VEC = {3, 5, 7}  # wxh pairs evicted on vector (rest scalar)


def _cs(c: int) -> int:
    return 120 * c if c < NCHUNK - 1 else S - C  # last chunk overlaps


def build_nc() -> bass.Bass:
    nc = bacc.Bacc("TRN2", target_bir_lowering=False)

    x0_d = nc.dram_tensor("x0", [128, SP], BF16, kind="ExternalInput")
    x1_d = nc.dram_tensor("x1", [128, SP], BF16, kind="ExternalInput")
    wqt_d = nc.dram_tensor("wqt", [128, 2, E], BF16, kind="ExternalInput")
    # cblob flat: [0:512]=wot, [512:896]=mask3, [896:1024]=ident
    cblob_d = nc.dram_tensor("cblob", [128, 1024], BF16, kind="ExternalInput")
    bvec_d = nc.dram_tensor("bvec", [128, 4], F32, kind="ExternalInput")
    out_d = nc.dram_tensor("out", [128, 2, S], BF16, kind="ExternalOutput")

    with tile.TileContext(nc) as tc:
        with (
            tc.tile_pool(name="const", bufs=1) as const,
            tc.tile_pool(name="work", bufs=4) as work,
            tc.tile_pool(name="grp", bufs=3) as grp,
            tc.tile_pool(name="ps", bufs=1, space="PSUM") as ps,
        ):
            # ---- DMA intake.
            x0_t = const.tile([128, SP], BF16)
            x1_t = const.tile([128, SP], BF16)
            cblob_t = const.tile([128, 1024], BF16)
            wqt_t = const.tile([128, 2, E], BF16)
            bvec_t = const.tile([128, 4], F32)
            # sync ring: wqt f0-half, mask3+id, x0 pieces 1-3, wot, x0 4-5
            nc.sync.dma_start(wqt_t[:, :, 0:128], wqt_d[:, :, 0:128])
            nc.sync.dma_start(cblob_t[:, 512:1024], cblob_d[:, 512:1024])
            for p0, w in X_PC[:3]:
                nc.sync.dma_start(x0_t[:, p0 : p0 + w], x0_d[:, p0 : p0 + w])
            nc.sync.dma_start(cblob_t[:, 0:512], cblob_d[:, 0:512])
            for p0, w in X_PC[3:]:
                nc.sync.dma_start(x0_t[:, p0 : p0 + w], x0_d[:, p0 : p0 + w])
            # scalar ring: bvec, x1p1, wqt f1-half, x1 rest
            nc.scalar.dma_start(bvec_t, bvec_d[:, :])
            p0, w = X_PC[0]
            nc.scalar.dma_start(x1_t[:, p0 : p0 + w], x1_d[:, p0 : p0 + w])
            nc.scalar.dma_start(wqt_t[:, :, 128:256], wqt_d[:, :, 128:256])
            for p0, w in X_PC[1:]:
                nc.scalar.dma_start(x1_t[:, p0 : p0 + w], x1_d[:, p0 : p0 + w])
            wot_t = cblob_t[:, 0:512]  # [:, e_i*256:(e_i+1)*256]
            mask3_t = cblob_t[:, 512:896]
            id_t = cblob_t[:, 896:1024]

            xs = [x0_t, x1_t]
            qtok0 = const.tile([128, SP], BF16)
            qtok1 = const.tile([128, SP], BF16)
            qtoks = [qtok0, qtok1]
            wxh_t = const.tile([128, NCHUNK, E], BF16)

            # persistent A slots; pad rows zeroed once
            A_slots = [
                const.tile([128, SOFT_G, H], BF16, name=f"A{i}") for i in range(3)
            ]
            for t in A_slots:
                nc.gpsimd.memset(t[96:128, :, :].bitcast(mybir.dt.uint32), 0)

            fo_tiles = {}
            pf_state = {}
            soft_state = {}

            def emit_wxh(c):
                ci = c % 2
                if ci == 0:
                    emit_wxh.pwx = ps.tile(
                        [128, 2, E], F32, tag="pswx", bufs=1, name="pwx"
                    )
                pwx = emit_wxh.pwx
                s0 = _cs(c)
                for e_i in range(2):
                    nc.tensor.matmul(
                        pwx[:, ci, :],
                        lhsT=xs[e_i][:, s0 : s0 + H],
                        rhs=wot_t[:, ts(e_i, 256)],
                        start=(e_i == 0),
                        stop=(e_i == 1),
                    )
                if ci == 1:
                    if (c // 2) in WXH_VEC:
                        nc.vector.tensor_copy(wxh_t[:, c - 1 : c + 1, :], pwx)
                    else:
                        nc.scalar.copy(wxh_t[:, c - 1 : c + 1, :], pwx)

            def emit_q(t_i):
                t0, w = T_CH[t_i]
                for f_i in range(2):
                    pq = ps.tile([128, 512], F32, tag="psq", bufs=2, name="pq")
                    for e_i in range(2):
                        nc.tensor.matmul(
                            pq[:, :w],
                            lhsT=wqt_t[:, e_i, ts(f_i, 128)],
                            rhs=xs[e_i][:, t0 : t0 + w],
                            start=(e_i == 0),
                            stop=(e_i == 1),
                        )
                    nc.vector.tensor_scalar_add(
                        qtoks[f_i][:, t0 : t0 + w],
                        pq[:, :w],
                        bvec_t[:, f_i : f_i + 1],
                    )

            def emit_e(k):
                pe_ = ps.tile([C, SOFT_G, H], F32, tag="pse", bufs=2, name="pe_")
                emit_e.pe[k] = pe_
                nc.tensor.matmul(
                    pe_[:, :, :],
                    lhsT=id_t[:, 0:C],
                    rhs=mask3_t[:, :],
                    start=True,
                    stop=False,
                )
                for gi in range(SOFT_G):
                    c = SOFT_G * k + gi
                    s0 = _cs(c)
                    for f_i in range(2):
                        nc.tensor.matmul(
                            pe_[:, gi, :],
                            lhsT=qtoks[f_i][:, s0 + PAD : s0 + PAD + C],
                            rhs=qtoks[f_i][:, s0 : s0 + H],
                            start=False,
                            stop=(f_i == 1 and gi == SOFT_G - 1),
                        )

            emit_e.pe = {}

            def emit_soft(k):
                pe_ = emit_e.pe.pop(k)
                A = A_slots[k % 3]
                soft_state[k] = A
                sums = work.tile([128, SOFT_G], F32, tag="sums", name="sums")
                nc.scalar.activation(
                    A[:C, :, :], pe_, mybir.ActivationFunctionType.Exp
                )
                nc.vector.tensor_reduce(
                    sums[:C, :],
                    A[:C, :, :],
                    axis=mybir.AxisListType.X,
                    op=mybir.AluOpType.add,
                )
                r = work.tile([128, SOFT_G], F32, tag="r", name="r")
                nc.vector.reciprocal(r[:C, :], sums[:C, :])
                nc.gpsimd.tensor_tensor(
                    out=A[:C, :, :],
                    in0=A[:C, :, :],
                    in1=r[:C, :, None].to_broadcast((C, SOFT_G, H)),
                    op=mybir.AluOpType.mult,
                )

            def emit_fin(c, at_ap):
                g = c // 2
                ci = c % 2
                if ci == 0:
                    pf_state[g] = ps.tile(
                        [128, 2, 2 * C], F32, tag="psf", bufs=2, name="pf"
                    )
                pf = pf_state[g]
                for f_i in range(2):
                    nc.tensor.matmul(
                        pf[:, f_i, ci * C : ci * C + C],
                        lhsT=wxh_t[:, c, ts(f_i, 128)],
                        rhs=at_ap,
                        start=True,
                        stop=True,
                    )
                if ci == 1:
                    fo = grp.tile([128, 2, 2 * C], BF16, tag="fo", name="fo")
                    fo_tiles[g] = fo
                    for f_i in range(2):
                        nc.scalar.activation(
                            fo[:, f_i, :],
                            pf[:, f_i, :],
                            mybir.ActivationFunctionType.Identity,
                            bias=bvec_t[:, 2 + f_i : 3 + f_i],
                            scale=1.0,
                        )
                    if g < 8:
                        nc.sync.dma_start(
                            out_d[:, :, 240 * g : 240 * g + 240], fo
                        )
                    else:
                        nc.sync.dma_start(out_d[:, :, 1920:2040], fo[:, :, 0:C])
                        nc.sync.dma_start(
                            out_d[:, :, 2040:2048], fo[:, :, 232:240]
                        )

            def emit_tfin(k):
                A = soft_state.pop(k)
                pat = ps.tile(
                    [128, SOFT_G, 128], BF16, tag="psat", bufs=1, name="pat"
                )
                for gi in range(SOFT_G):
                    nc.tensor.transpose(pat[:, gi, :], A[:, gi, :], id_t)
                at = work.tile([128, SOFT_G, 128], BF16, tag="at", name="at")
                nc.vector.tensor_copy(at, pat)
                for gi in range(SOFT_G):
                    emit_fin(SOFT_G * k + gi, at[:, gi, 0:C])

            # ---- schedule: qtok/energy ASAP, softmax chains right after,
            # transpose+fin deferred, wxh as PE filler.
            emit_q(0)
            emit_q(1)
            emit_e(0)
            emit_soft(0)
            emit_wxh(0)
            emit_q(2)
            emit_e(1)
            emit_soft(1)
            emit_wxh(1)
            emit_wxh(2)
            emit_q(3)
            emit_e(2)
            emit_soft(2)
            emit_wxh(3)
            emit_tfin(0)
            emit_e(3)
            emit_soft(3)
            emit_wxh(4)
            emit_wxh(5)
            emit_q(4)
            emit_e(4)
            emit_tfin(1)
            emit_soft(4)
            emit_wxh(6)
            emit_wxh(7)
            emit_q(5)
            emit_e(5)
            emit_wxh(8)
            emit_wxh(9)
            emit_tfin(2)
            emit_soft(5)
            emit_wxh(10)
            emit_wxh(11)
            emit_wxh(12)
            emit_wxh(13)
            emit_tfin(3)
            emit_wxh(14)
            emit_wxh(15)
            emit_wxh(16)
            emit_wxh(17)
            emit_tfin(4)
            emit_tfin(5)

    nc.compile()
    return nc


def make_in_maps(x, Wq, bq, Wo, bo):
    x = np.asarray(x, dtype=np.float32)
    Wq = np.asarray(Wq, dtype=np.float32)
    bq = np.asarray(bq, dtype=np.float32)
    Wo = np.asarray(Wo, dtype=np.float32)
    bo = np.asarray(bo, dtype=np.float32)

    bf = ml_dtypes.bfloat16
    f = 1.0 / np.sqrt(np.sqrt(E) * 1.5)  # 1/sqrt(24) folded into Wq, bq
    wqt = np.ascontiguousarray(
        (Wq * f).T.reshape(2, 128, E).transpose(1, 0, 2)
    ).astype(bf)
    wot = np.ascontiguousarray(
        (Wo / KERNEL).T.reshape(2, 128, E).transpose(1, 0, 2)
    ).astype(bf)

    # cblob flat: [0:512]=wot, [512:896]=mask3 (3x), [896:1024]=ident
    cblob = np.zeros((128, 1024), dtype=bf)
    cblob[:, 0:256] = wot[:, 0, :]
    cblob[:, 256:512] = wot[:, 1, :]
    mask = np.full((128, 128), NEG, dtype=np.float32)
    for c in range(128):
        mask[c, c : min(c + KERNEL, 128)] = 0.0
    for gi in range(3):
        cblob[:, 512 + 128 * gi : 640 + 128 * gi] = mask.astype(bf)
    cblob[:, 896:1024] = np.eye(128, dtype=bf)

    bvec = np.zeros((128, 4), dtype=np.float32)
    bvec[:, 0:2] = (bq * f).reshape(2, 128).T
    bvec[:, 2:4] = (bo / KERNEL).reshape(2, 128).T

    in_maps = []
    for b in range(B):
        xp = np.zeros((E, SP), dtype=np.float32)
        xp[:, PAD : PAD + S] = x[b]
        xpb = xp.astype(bf)
        in_maps.append(
            dict(
                x0=np.ascontiguousarray(xpb[0:128]),
                x1=np.ascontiguousarray(xpb[128:256]),
                wqt=wqt,
                cblob=cblob,
                bvec=bvec,
            )
        )
    return in_maps


_NC_CACHE = {}


def kernel(x, Wq, bq, Wo, bo):
    res = kernel_with_results(x, Wq, bq, Wo, bo)
    outs = []
    for r in res.results:
        o = np.asarray(r["out"])  # [128, 2, S] bf16
        outs.append(o.transpose(1, 0, 2).reshape(E, S).astype(np.float32))
    return np.stack(outs)


def kernel_with_results(x, Wq, bq, Wo, bo, trace=False, **kwargs):
    in_maps = make_in_maps(x, Wq, bq, Wo, bo)
    if "nc" not in _NC_CACHE:
        _NC_CACHE["nc"] = build_nc()
    return run_bass_kernel_spmd(
        _NC_CACHE["nc"], in_maps, core_ids=list(range(B)), trace=trace, **kwargs
    )
